# revision 3
# baseline (speedup 1.0000x reference)
"""Trainium2 Bass kernel for nn_Attention_386547057357 (Transformer-XL style
relative-position sparse attention).

Sharding: data-parallel over batch — core c computes batch element c.
All weights replicated per core.

Math (per batch element):
    X = [memory; x]  (1024, 512)
    q = x @ W_q  (256, 512);  k = X @ W_k;  v = X @ W_v
    qhat = q + u_emb (per head);  qtld = q + v_emb
    RW = R @ W_rel  (1024, 512)     [host constant x W_rel, computed on host]
    ac[n, m]  = qhat_h[n] . k_h[m]            (= term_a + term_c)
    bd[n, r'] = qtld_h[n] . RW[1023 - r']     (= term_b + term_d, reversed r)
    scores[n, m] = (ac[n, m] + bd[n, 255 - n + m]) * scale  + causal mask
    out = softmax(scores) @ v @ W_out + b_out

v2 implementation notes:
  - RW^T computed on host (bf16), shipped as input: kills 8 fp32-mode PE
    matmuls from v1.
  - bd shift via bf16 DRAM round trip: write RAW bd logits bf16 with row
    stride 1281 (cols [1024:1281] pre-written once to -1e38 = causal mask),
    read back with diagonal AP [[1280, 128], [1, 1024]] at base offset 255.
  - the shifted bd is merged into the ac PSUM with an identity-matrix
    matmul (PE accumulate) — no DVE add pass.
  - single ACT Exp(scale*(ac+bd)) with fused accum_out rowsum — no DVE
    reduce pass.
  - softmax normalize + bf16 cast via gpsimd.normalize_recip.
  - attn transpose for the PV matmul via SBUF-source dma_gather(transpose).
  - const DMA loads ordered so the PE can start (qT) after ~0.8 MB.
"""

import sys

sys.path.insert(0, "/opt/trn_rl_repo")

import numpy as np
import ml_dtypes

import concourse.bass as bass
import concourse.mybir as mybir
import concourse.tile as tile
from concourse import bacc, library_config
from concourse.bass_utils import run_bass_kernel_spmd
from concourse.tile_rust import add_dep_helper

BF16 = ml_dtypes.bfloat16
F32 = np.float32

DIM = 512
NHEAD = 8
DHEAD = 64
CTX = 1024
NOCT = 11
B = 8
SEQ = 256
MEM = 768
TOT = MEM + SEQ  # 1024
SCALE = DHEAD ** -0.5  # 0.125
RSTRIDE = 1281  # bd scratch row stride (1024 data + 257 mask pad)
PAD = -1e38     # mask logit: exp(SCALE*(ac+PAD)) == 0

dt = mybir.dt
AF = mybir.ActivationFunctionType
ALU = mybir.AluOpType


# ---------------------------------------------------------------- host consts
def _positional_encoding():
    coords = np.arange(CTX, dtype=F32)[:, None]
    octaves = np.arange(1 - NOCT, 1, dtype=F32)
    mult = ((2.0 ** octaves) * np.pi).astype(F32)
    scaled = (coords * mult[None, :]).astype(F32)
    return np.concatenate([np.sin(scaled), np.cos(scaled)], axis=-1).astype(F32)


def _chunked(w, nchunk):
    """(128*nchunk, F) -> (128, nchunk, F) with [p, c, f] = w[128c + p, f]."""
    f = w.shape[1]
    return np.ascontiguousarray(w.reshape(nchunk, 128, f).transpose(1, 0, 2))


# ---------------------------------------------------------------- bass program
def build_program():
    nc = bacc.Bacc("TRN2", target_bir_lowering=False, debug=False)

    xt_d = nc.dram_tensor("xt", [128, 4, TOT], dt.bfloat16, kind="ExternalInput")
    wqkv_d = nc.dram_tensor("wqkv", [128, 4, 1536], dt.bfloat16, kind="ExternalInput")
    rwt_d = nc.dram_tensor("rwt", [128, 4, CTX], dt.bfloat16, kind="ExternalInput")
    wout_d = nc.dram_tensor("wout", [128, 4, 512], dt.bfloat16, kind="ExternalInput")
    bout_d = nc.dram_tensor("bout", [128, 512], dt.float32, kind="ExternalInput")
    u2_d = nc.dram_tensor("u2", [128, 1], dt.float32, kind="ExternalInput")
    v2_d = nc.dram_tensor("v2", [128, 1], dt.float32, kind="ExternalInput")
    gidx_d = nc.dram_tensor("gidx", [128, 16], dt.int16, kind="ExternalInput")
    ident_d = nc.dram_tensor("ident", [128, 128], dt.bfloat16, kind="ExternalInput")
    zpad_d = nc.dram_tensor("zpad", [128, RSTRIDE - TOT], dt.bfloat16,
                            kind="ExternalInput")
    out_d = nc.dram_tensor("out", [SEQ, 512], dt.float32, kind="ExternalOutput")

    with tile.TileContext(nc) as tc:
        _body(tc, xt_d, wqkv_d, rwt_d, wout_d, bout_d, u2_d, v2_d, gidx_d,
              ident_d, zpad_d, out_d)
    nc.compile()
    return nc


def _body(tc, xt_d, wqkv_d, rwt_d, wout_d, bout_d, u2_d, v2_d, gidx_d,
          ident_d, zpad_d, out_d):
    nc = tc.nc
    from contextlib import ExitStack

    with ExitStack() as ctx:
        consts = ctx.enter_context(tc.tile_pool(name="consts", bufs=1))

        # ---- constants / weights, ordered so first PE work unblocks early
        u2 = consts.tile([128, 1], dt.float32)
        nc.sync.dma_start(u2[:], u2_d.ap())
        v2 = consts.tile([128, 1], dt.float32)
        nc.sync.dma_start(v2[:], v2_d.ap())
        gidx = consts.tile([128, 16], dt.int16)
        nc.sync.dma_start(gidx[:], gidx_d.ap())
        ident = consts.tile([128, 128], dt.bfloat16)
        nc.sync.dma_start(ident[:], ident_d.ap())
        zpad = consts.tile([128, RSTRIDE - TOT], dt.bfloat16)
        nc.sync.dma_start(zpad[:], zpad_d.ap())

        xt = consts.tile([128, 4, TOT], dt.bfloat16)
        wqkv = consts.tile([128, 4, 1536], dt.bfloat16)
        # x part + W_q first (unblocks qT), then rwt (bd), then k/m, v, out
        nc.sync.dma_start(xt[:, :, MEM:TOT], xt_d.ap()[:, :, MEM:TOT])
        nc.sync.dma_start(wqkv[:, :, 0:512], wqkv_d.ap()[:, :, 0:512])
        rwt = consts.tile([128, 4, CTX], dt.bfloat16)
        nc.sync.dma_start(rwt[:], rwt_d.ap())
        nc.sync.dma_start(wqkv[:, :, 512:1024], wqkv_d.ap()[:, :, 512:1024])
        nc.sync.dma_start(xt[:, :, 0:MEM], xt_d.ap()[:, :, 0:MEM])
        nc.sync.dma_start(wqkv[:, :, 1024:1536], wqkv_d.ap()[:, :, 1024:1536])
        wout = consts.tile([128, 4, 512], dt.bfloat16)
        nc.sync.dma_start(wout[:], wout_d.ap())
        bout = consts.tile([128, 512], dt.float32)
        nc.sync.dma_start(bout[:], bout_d.ap())

        # persistent intermediates
        qhatT = consts.tile([128, 4, SEQ], dt.bfloat16)  # (q+u)^T  [hd, n]
        qtldT = consts.tile([128, 4, SEQ], dt.bfloat16)  # (q+v)^T  [hd, n]
        kT = consts.tile([128, 4, TOT], dt.bfloat16)     # k^T      [hd, m]
        vv = consts.tile([128, 8, 512], dt.bfloat16)     # V        [m, hd]
        avt = consts.tile([128, 4, SEQ], dt.bfloat16)    # attnV^T  [hd, n]

        with (
            tc.tile_pool(name="mps", bufs=3, space="PSUM") as mps,
            tc.tile_pool(name="qps_sb", bufs=2) as qps_sb,
            tc.tile_pool(name="pvps", bufs=2, space="PSUM") as pvps,
            tc.tile_pool(name="hsb", bufs=4) as hsb,
            tc.tile_pool(name="ebp", bufs=6) as ebp,
            tc.tile_pool(name="afp", bufs=3) as afp,
            tc.tile_pool(name="rsp", bufs=4) as rsp,
            tc.tile_pool(name="atp", bufs=3) as atp,
            tc.tile_pool(name="bdd", bufs=8, space="DRAM") as bddp,
        ):
            lib_inst = nc.gpsimd.load_library(library_config.mlp)

            # per-head DRAM scratch; mask pads written ONCE up front
            bdds = [bddp.tile([SEQ, RSTRIDE], dt.bfloat16, tag="bdd",
                              name=f"bdd{h}")
                    for h in range(NHEAD)]
            for h in range(NHEAD):
                for a in range(2):
                    nc.sync.dma_start(
                        bdds[h][128 * a:128 * (a + 1), TOT:RSTRIDE], zpad[:])

            # ---------------- stage emitters
            def emit_qt():
                for hp in range(4):
                    psw = mps.tile([128, 1024], dt.float32, tag="m")
                    ps = psw[:, 0:SEQ]
                    for ch in range(4):
                        nc.tensor.matmul(ps, wqkv[:, ch, 128 * hp:128 * (hp + 1)],
                                         xt[:, ch, MEM:TOT],
                                         start=(ch == 0), stop=(ch == 3))
                    # all-f32 tensor_scalar, then cast-copy to bf16 (mixed-
                    # dtype multi-operand DVE ops fault on this runtime)
                    qf = qps_sb.tile([128, SEQ], dt.float32, tag="qf")
                    nc.vector.tensor_scalar_add(qf[:], ps, u2[:])
                    nc.vector.tensor_copy(qhatT[:, hp, :], qf[:])
                    qf2 = qps_sb.tile([128, SEQ], dt.float32, tag="qf")
                    nc.vector.tensor_scalar_add(qf2[:], ps, v2[:])
                    nc.vector.tensor_copy(qtldT[:, hp, :], qf2[:])

            def emit_kt(kt_hps):
                for hp in kt_hps:
                    ps = mps.tile([128, 1024], dt.float32, tag="m")
                    for mh in range(2):
                        for ch in range(4):
                            nc.tensor.matmul(
                                ps[:, 512 * mh:512 * (mh + 1)],
                                wqkv[:, ch, 512 + 128 * hp:512 + 128 * (hp + 1)],
                                xt[:, ch, 512 * mh:512 * (mh + 1)],
                                start=(ch == 0), stop=(ch == 3))
                    nc.vector.tensor_copy(kT[:, hp, :], ps[:])

            def emit_v(v_mc0s):
                for mc0 in v_mc0s:
                    ps = mps.tile([128, 1024], dt.float32, tag="m")
                    for k2 in range(2):
                        mc = mc0 + k2
                        for ch in range(4):
                            nc.tensor.matmul(
                                ps[:, 512 * k2:512 * (k2 + 1)],
                                xt[:, ch, 128 * mc:128 * (mc + 1)],
                                wqkv[:, ch, 1024:1536],
                                start=(ch == 0), stop=(ch == 3))
                    nc.vector.tensor_copy(vv[:, mc0:mc0 + 2, :], ps[:])

            ebds = {}

            def emit_bd(h):
                # raw bd logits -> bf16 -> DRAM rows; then shifted readback
                hp, par = h // 2, h % 2
                pb = 64 * par
                for n2 in range(2):
                    ps = mps.tile([128, 1024], dt.float32, tag="m")
                    for rh in range(2):
                        nc.tensor.matmul(
                            ps[:, 512 * rh:512 * (rh + 1)],
                            qtldT[pb:pb + 64, hp, 128 * n2:128 * (n2 + 1)],
                            rwt[pb:pb + 64, hp, 512 * rh:512 * (rh + 1)],
                            start=True, stop=True)
                    bdt = hsb.tile([128, TOT], dt.bfloat16, tag="bdt")
                    nc.vector.tensor_copy(bdt[:], ps[:])
                    nc.sync.dma_start(
                        bdds[h][128 * n2:128 * (n2 + 1), 0:TOT], bdt[:])
                for n2 in range(2):
                    t = ebp.tile([128, TOT], dt.bfloat16, tag="ebd")
                    src = bass.AP(
                        bdds[h].tensor,
                        bdds[h].offset + 255 + n2 * 128 * (RSTRIDE - 1),
                        [[RSTRIDE - 1, 128], [1, TOT]])
                    nc.sync.dma_start(t[:], src)
                    ebds[(h, n2)] = t

            attns = {}
            attnTs = {}

            def emit_ac(h):
                # ac matmuls + identity-add of shifted bd -> exp(+rowsum)
                # -> normalize_recip -> gather transpose
                hp, par = h // 2, h % 2
                pb = 64 * par
                attn = atp.tile([128, 2, TOT], dt.bfloat16, tag="attn")
                attns[h] = attn
                for n2 in range(2):
                    ps = mps.tile([128, 1024], dt.float32, tag="m")
                    ebd = ebds[(h, n2)]
                    for mh in range(2):
                        sl = slice(512 * mh, 512 * (mh + 1))
                        nc.tensor.matmul(
                            ps[:, sl],
                            qhatT[pb:pb + 64, hp, 128 * n2:128 * (n2 + 1)],
                            kT[pb:pb + 64, hp, sl],
                            start=True, stop=False)
                        nc.tensor.matmul(ps[:, sl], ident[:], ebd[:, sl],
                                         start=False, stop=True)
                    af = afp.tile([128, TOT], dt.float32, tag="af")
                    rs = rsp.tile([128, 1], dt.float32, tag="rs")
                    nc.scalar.activation(af[:], ps[:], func=AF.Exp,
                                         scale=SCALE, accum_out=rs[:])
                    nc.gpsimd.normalize_recip(attn[:, n2, :], af[:], rs[:])
                attnT = atp.tile([128, 8, SEQ], dt.bfloat16, tag="attnT")
                attnTs[h] = attnT
                g = nc.gpsimd.dma_gather(
                    out_ap=attnT[:], in_ap=attn[:], idxs_ap=gidx[:],
                    num_idxs=SEQ, num_idxs_reg=SEQ, elem_size=TOT,
                    transpose=True, sbuf_tokens_per_rank=128,
                    sbuf_free_dim_per_rank=2 * TOT,
                    sbuf_free_dim_pad_per_rank=0, sbuf_byte_offset=0)
                add_dep_helper(g.ins, lib_inst.ins,
                               reason="dma_gather needs mlp gpsimd library")

            def emit_pv(h0):
                # PV for head pair (h0, h0+1), col-tiled into one PSUM tile
                hp = h0 // 2
                pvt = pvps.tile([128, SEQ], dt.float32, tag="pv")
                for par in range(2):
                    h = h0 + par
                    pb = 64 * par
                    for mc in range(8):
                        nc.tensor.matmul(
                            pvt[pb:pb + 64, :],
                            vv[:, mc, 64 * h:64 * (h + 1)],
                            attnTs[h][:, mc, :],
                            start=(mc == 0), stop=(mc == 7),
                            tile_position=(0, pb))
                    nc.vector.tensor_copy(avt[pb:pb + 64, hp, :],
                                          pvt[pb:pb + 64, :])

            def emit_out():
                for n2 in range(2):
                    psw = mps.tile([128, 1024], dt.float32, tag="m")
                    ps = psw[:, 0:512]
                    for c4 in range(4):
                        nc.tensor.matmul(ps,
                                         avt[:, c4, 128 * n2:128 * (n2 + 1)],
                                         wout[:, c4, :],
                                         start=(c4 == 0), stop=(c4 == 3))
                    osb = hsb.tile([128, 512], dt.float32, tag="osb")
                    nc.vector.tensor_add(osb[:], ps, bout[:])
                    nc.sync.dma_start(out_d.ap()[128 * n2:128 * (n2 + 1), :],
                                      osb[:])

            # ---------------- software-pipelined emission
            emit_qt()
            emit_bd(0); emit_bd(1)
            emit_kt((0, 1))          # covers h0/h1 round trip
            emit_bd(2); emit_bd(3)
            emit_ac(0); emit_ac(1)
            emit_kt((2, 3))
            emit_bd(4); emit_bd(5)
            emit_ac(2); emit_ac(3)
            emit_v((0, 2))
            emit_bd(6); emit_bd(7)
            emit_ac(4); emit_ac(5)
            emit_v((4, 6))
            emit_pv(0)
            emit_ac(6); emit_ac(7)
            emit_pv(2); emit_pv(4); emit_pv(6)
            emit_out()


# ---------------------------------------------------------------- host wrapper
_PROGRAM = None


def _get_program():
    global _PROGRAM
    if _PROGRAM is None:
        _PROGRAM = build_program()
    return _PROGRAM


def make_in_maps(x, memory, W_qkv, W_rel, W_out, b_out, u_emb, v_emb):
    x = np.asarray(x, dtype=F32)
    memory = np.asarray(memory, dtype=F32)
    W_qkv = np.asarray(W_qkv, dtype=F32)
    W_rel = np.asarray(W_rel, dtype=F32)
    W_out = np.asarray(W_out, dtype=F32)
    b_out = np.asarray(b_out, dtype=F32)
    u_emb = np.asarray(u_emb, dtype=F32)
    v_emb = np.asarray(v_emb, dtype=F32)

    R = _positional_encoding()                       # (1024, 22)
    rw = R @ W_rel                                   # (1024, 512)
    rwt = _chunked(np.ascontiguousarray(rw[::-1].T), 4).astype(BF16)

    wqkv = _chunked(W_qkv, 4).astype(BF16)           # (128, 4, 1536)
    wout = _chunked(W_out, 4).astype(BF16)           # (128, 4, 512)
    bout = np.tile(b_out[None, :], (128, 1)).astype(F32)
    u2 = np.tile(u_emb, 2)[:, None].astype(F32)
    v2 = np.tile(v_emb, 2)[:, None].astype(F32)
    p = np.arange(128)[:, None] % 16
    s = np.arange(16)[None, :]
    gidx = (s * 16 + p).astype(np.int16)             # (128, 16)
    ident = np.eye(128).astype(BF16)
    zpad = np.full((128, RSTRIDE - TOT), PAD, dtype=F32).astype(BF16)

    shared = dict(wqkv=wqkv, rwt=rwt, wout=wout, bout=bout,
                  u2=u2, v2=v2, gidx=gidx, ident=ident, zpad=zpad)
    in_maps = []
    for c in range(B):
        X = np.concatenate([memory[c], x[c]], axis=0)          # (1024, 512)
        xt = _chunked(np.ascontiguousarray(X.T), 4).astype(BF16)  # (128,4,1024)
        in_maps.append(dict(xt=xt, **shared))
    return in_maps


def run(in_maps, trace=False, **kw):
    nc = _get_program()
    res = run_bass_kernel_spmd(nc, in_maps, core_ids=list(range(B)),
                               trace=trace, **kw)
    out = np.stack([res.results[c]["out"] for c in range(B)]).astype(F32)
    return out, res


def kernel(x, memory, W_qkv, W_rel, W_out, b_out, u_emb, v_emb):
    in_maps = make_in_maps(x, memory, W_qkv, W_rel, W_out, b_out, u_emb, v_emb)
    out, _ = run(in_maps)
    return out.reshape(B, SEQ, DIM)


# revision 4
# speedup vs baseline: 1.8297x; 1.8297x over previous
"""Trainium2 Bass kernel for nn_Attention_386547057357 (Transformer-XL style
relative-position sparse attention).

Sharding: data-parallel over batch — core c computes batch element c.
All weights replicated per core.

Math (per batch element):
    X = [memory; x]  (1024, 512)
    q = x @ W_q  (256, 512);  k = X @ W_k;  v = X @ W_v
    qhat = q + u_emb (per head);  qtld = q + v_emb
    RW = R @ W_rel  (1024, 512)     [host constant x W_rel, computed on host]
    ac[n, m]  = qhat_h[n] . k_h[m]            (= term_a + term_c)
    bd[n, r'] = qtld_h[n] . RW[1023 - r']     (= term_b + term_d, reversed r)
    scores[n, m] = (ac[n, m] + bd[n, 255 - n + m]) * scale  + causal mask
    out = softmax(scores) @ v @ W_out + b_out

v2 implementation notes:
  - RW^T computed on host (bf16), shipped as input: kills 8 fp32-mode PE
    matmuls from v1.
  - bd shift via bf16 DRAM round trip: write RAW bd logits bf16 with row
    stride 1281 (cols [1024:1281] pre-written once to -1e38 = causal mask),
    read back with diagonal AP [[1280, 128], [1, 1024]] at base offset 255.
  - the shifted bd is merged into the ac PSUM with an identity-matrix
    matmul (PE accumulate) — no DVE add pass.
  - single ACT Exp(scale*(ac+bd)) with fused accum_out rowsum — no DVE
    reduce pass.
  - softmax normalize + bf16 cast via gpsimd.normalize_recip.
  - attn transpose for the PV matmul via SBUF-source dma_gather(transpose).
  - const DMA loads ordered so the PE can start (qT) after ~0.8 MB.
"""

import sys

sys.path.insert(0, "/opt/trn_rl_repo")

import numpy as np
import ml_dtypes

import concourse.bass as bass
import concourse.mybir as mybir
import concourse.tile as tile
from concourse import bacc, library_config
from concourse.bass_utils import run_bass_kernel_spmd
from concourse.tile_rust import add_dep_helper

BF16 = ml_dtypes.bfloat16
F32 = np.float32

DIM = 512
NHEAD = 8
DHEAD = 64
CTX = 1024
NOCT = 11
B = 8
SEQ = 256
MEM = 768
TOT = MEM + SEQ  # 1024
SCALE = DHEAD ** -0.5  # 0.125
RSTRIDE = 1281  # bd scratch row stride (1024 data + 257 mask pad)
PAD = -1e38     # mask logit: exp(SCALE*(ac+PAD)) == 0

dt = mybir.dt
AF = mybir.ActivationFunctionType
ALU = mybir.AluOpType


# ---------------------------------------------------------------- host consts
def _positional_encoding():
    coords = np.arange(CTX, dtype=F32)[:, None]
    octaves = np.arange(1 - NOCT, 1, dtype=F32)
    mult = ((2.0 ** octaves) * np.pi).astype(F32)
    scaled = (coords * mult[None, :]).astype(F32)
    return np.concatenate([np.sin(scaled), np.cos(scaled)], axis=-1).astype(F32)


def _chunked(w, nchunk):
    """(128*nchunk, F) -> (128, nchunk, F) with [p, c, f] = w[128c + p, f]."""
    f = w.shape[1]
    return np.ascontiguousarray(w.reshape(nchunk, 128, f).transpose(1, 0, 2))


# ---------------------------------------------------------------- bass program
def build_program():
    nc = bacc.Bacc("TRN2", target_bir_lowering=False, debug=False)

    xt_d = nc.dram_tensor("xt", [128, 4, TOT], dt.bfloat16, kind="ExternalInput")
    wqkv_d = nc.dram_tensor("wqkv", [128, 4, 1536], dt.bfloat16, kind="ExternalInput")
    rwt_d = nc.dram_tensor("rwt", [128, 4, CTX], dt.bfloat16, kind="ExternalInput")
    wout_d = nc.dram_tensor("wout", [128, 4, 512], dt.bfloat16, kind="ExternalInput")
    bout_d = nc.dram_tensor("bout", [128, 512], dt.float32, kind="ExternalInput")
    u2_d = nc.dram_tensor("u2", [128, 1], dt.float32, kind="ExternalInput")
    v2_d = nc.dram_tensor("v2", [128, 1], dt.float32, kind="ExternalInput")
    gidx_d = nc.dram_tensor("gidx", [128, 16], dt.int16, kind="ExternalInput")
    ident_d = nc.dram_tensor("ident", [128, 128], dt.bfloat16, kind="ExternalInput")
    zpad_d = nc.dram_tensor("zpad", [128, RSTRIDE - TOT], dt.bfloat16,
                            kind="ExternalInput")
    out_d = nc.dram_tensor("out", [SEQ, 512], dt.float32, kind="ExternalOutput")

    with tile.TileContext(nc) as tc:
        _body(tc, xt_d, wqkv_d, rwt_d, wout_d, bout_d, u2_d, v2_d, gidx_d,
              ident_d, zpad_d, out_d)
    nc.compile()
    return nc


def _body(tc, xt_d, wqkv_d, rwt_d, wout_d, bout_d, u2_d, v2_d, gidx_d,
          ident_d, zpad_d, out_d):
    nc = tc.nc
    from contextlib import ExitStack

    with ExitStack() as ctx:
        consts = ctx.enter_context(tc.tile_pool(name="consts", bufs=1))

        # ---- constants / weights, ordered so first PE work unblocks early
        u2 = consts.tile([128, 1], dt.float32)
        nc.sync.dma_start(u2[:], u2_d.ap())
        v2 = consts.tile([128, 1], dt.float32)
        nc.sync.dma_start(v2[:], v2_d.ap())
        gidx = consts.tile([128, 16], dt.int16)
        nc.sync.dma_start(gidx[:], gidx_d.ap())
        ident = consts.tile([128, 128], dt.bfloat16)
        nc.sync.dma_start(ident[:], ident_d.ap())
        zpad = consts.tile([128, RSTRIDE - TOT], dt.bfloat16)
        nc.sync.dma_start(zpad[:], zpad_d.ap())

        xt = consts.tile([128, 4, TOT], dt.bfloat16)
        wqkv = consts.tile([128, 4, 1536], dt.bfloat16)
        # x part + W_q first (unblocks qT), then rwt (bd), then k/m, v, out
        nc.sync.dma_start(xt[:, :, MEM:TOT], xt_d.ap()[:, :, MEM:TOT])
        nc.sync.dma_start(wqkv[:, :, 0:512], wqkv_d.ap()[:, :, 0:512])
        rwt = consts.tile([128, 4, CTX], dt.bfloat16)
        nc.sync.dma_start(rwt[:], rwt_d.ap())
        nc.sync.dma_start(wqkv[:, :, 512:1024], wqkv_d.ap()[:, :, 512:1024])
        nc.sync.dma_start(xt[:, :, 0:MEM], xt_d.ap()[:, :, 0:MEM])
        nc.sync.dma_start(wqkv[:, :, 1024:1536], wqkv_d.ap()[:, :, 1024:1536])
        wout = consts.tile([128, 4, 512], dt.bfloat16)
        nc.sync.dma_start(wout[:], wout_d.ap())
        bout = consts.tile([128, 512], dt.float32)
        nc.sync.dma_start(bout[:], bout_d.ap())

        # persistent intermediates
        qhatT = consts.tile([128, 4, SEQ], dt.bfloat16)  # (q+u)^T  [hd, n]
        qtldT = consts.tile([128, 4, SEQ], dt.bfloat16)  # (q+v)^T  [hd, n]
        kT = consts.tile([128, 4, TOT], dt.bfloat16)     # k^T      [hd, m]
        vv = consts.tile([128, 8, 512], dt.bfloat16)     # V        [m, hd]
        avt = consts.tile([128, 4, SEQ], dt.bfloat16)    # attnV^T  [hd, n]

        with (
            tc.tile_pool(name="mps", bufs=3, space="PSUM") as mps,
            tc.tile_pool(name="qps_sb", bufs=2) as qps_sb,
            tc.tile_pool(name="pvps", bufs=2, space="PSUM") as pvps,
            tc.tile_pool(name="hsb", bufs=4) as hsb,
            tc.tile_pool(name="ebp", bufs=6) as ebp,
            tc.tile_pool(name="afp", bufs=3) as afp,
            tc.tile_pool(name="rsp", bufs=4) as rsp,
            tc.tile_pool(name="atp", bufs=3) as atp,
            tc.tile_pool(name="bdd", bufs=8, space="DRAM") as bddp,
        ):
            lib_inst = nc.gpsimd.load_library(library_config.mlp)

            # per-head DRAM scratch; mask pads written ONCE up front
            bdds = [bddp.tile([SEQ, RSTRIDE], dt.bfloat16, tag="bdd",
                              name=f"bdd{h}")
                    for h in range(NHEAD)]
            for h in range(NHEAD):
                for a in range(2):
                    nc.sync.dma_start(
                        bdds[h][128 * a:128 * (a + 1), TOT:RSTRIDE], zpad[:])

            # ---------------- stage emitters
            def emit_qt():
                for hp in range(4):
                    psw = mps.tile([128, 1024], dt.float32, tag="m")
                    ps = psw[:, 0:SEQ]
                    for ch in range(4):
                        nc.tensor.matmul(ps, wqkv[:, ch, 128 * hp:128 * (hp + 1)],
                                         xt[:, ch, MEM:TOT],
                                         start=(ch == 0), stop=(ch == 3))
                    # all-f32 tensor_scalar, then cast-copy to bf16 (mixed-
                    # dtype multi-operand DVE ops fault on this runtime)
                    qf = qps_sb.tile([128, SEQ], dt.float32, tag="qf")
                    nc.vector.tensor_scalar_add(qf[:], ps, u2[:])
                    nc.vector.tensor_copy(qhatT[:, hp, :], qf[:])
                    qf2 = qps_sb.tile([128, SEQ], dt.float32, tag="qf")
                    nc.vector.tensor_scalar_add(qf2[:], ps, v2[:])
                    nc.vector.tensor_copy(qtldT[:, hp, :], qf2[:])

            def emit_kt(kt_hps):
                for hp in kt_hps:
                    ps = mps.tile([128, 1024], dt.float32, tag="m")
                    for mh in range(2):
                        for ch in range(4):
                            nc.tensor.matmul(
                                ps[:, 512 * mh:512 * (mh + 1)],
                                wqkv[:, ch, 512 + 128 * hp:512 + 128 * (hp + 1)],
                                xt[:, ch, 512 * mh:512 * (mh + 1)],
                                start=(ch == 0), stop=(ch == 3))
                    nc.vector.tensor_copy(kT[:, hp, :], ps[:])

            def emit_v(v_mc0s):
                for mc0 in v_mc0s:
                    ps = mps.tile([128, 1024], dt.float32, tag="m")
                    for k2 in range(2):
                        mc = mc0 + k2
                        for ch in range(4):
                            nc.tensor.matmul(
                                ps[:, 512 * k2:512 * (k2 + 1)],
                                xt[:, ch, 128 * mc:128 * (mc + 1)],
                                wqkv[:, ch, 1024:1536],
                                start=(ch == 0), stop=(ch == 3))
                    nc.vector.tensor_copy(vv[:, mc0:mc0 + 2, :], ps[:])

            ebds = {}

            def emit_bd(h):
                # raw bd logits -> bf16 -> DRAM rows; then shifted readback
                hp, par = h // 2, h % 2
                pb = 64 * par
                for n2 in range(2):
                    ps = mps.tile([128, 1024], dt.float32, tag="m")
                    for rh in range(2):
                        nc.tensor.matmul(
                            ps[:, 512 * rh:512 * (rh + 1)],
                            qtldT[pb:pb + 64, hp, 128 * n2:128 * (n2 + 1)],
                            rwt[pb:pb + 64, hp, 512 * rh:512 * (rh + 1)],
                            start=True, stop=True)
                    bdt = hsb.tile([128, TOT], dt.bfloat16, tag="bdt")
                    nc.vector.tensor_copy(bdt[:], ps[:])
                    nc.sync.dma_start(
                        bdds[h][128 * n2:128 * (n2 + 1), 0:TOT], bdt[:])
                for n2 in range(2):
                    t = ebp.tile([128, TOT], dt.bfloat16, tag="ebd")
                    src = bass.AP(
                        bdds[h].tensor,
                        bdds[h].offset + 255 + n2 * 128 * (RSTRIDE - 1),
                        [[RSTRIDE - 1, 128], [1, TOT]])
                    nc.sync.dma_start(t[:], src)
                    ebds[(h, n2)] = t

            attns = {}
            attnTs = {}

            def emit_ac(h):
                # ac matmuls + identity-add of shifted bd -> exp(+rowsum)
                # -> normalize_recip -> gather transpose
                hp, par = h // 2, h % 2
                pb = 64 * par
                attn = atp.tile([128, 2, TOT], dt.bfloat16, tag="attn")
                attns[h] = attn
                for n2 in range(2):
                    ps = mps.tile([128, 1024], dt.float32, tag="m")
                    ebd = ebds[(h, n2)]
                    for mh in range(2):
                        sl = slice(512 * mh, 512 * (mh + 1))
                        nc.tensor.matmul(
                            ps[:, sl],
                            qhatT[pb:pb + 64, hp, 128 * n2:128 * (n2 + 1)],
                            kT[pb:pb + 64, hp, sl],
                            start=True, stop=False)
                        nc.tensor.matmul(ps[:, sl], ident[:], ebd[:, sl],
                                         start=False, stop=True)
                    af = afp.tile([128, TOT], dt.float32, tag="af")
                    rs = rsp.tile([128, 1], dt.float32, tag="rs")
                    nc.scalar.activation(af[:], ps[:], func=AF.Exp,
                                         scale=SCALE, accum_out=rs[:])
                    rec = rsp.tile([128, 1], dt.float32, tag="rec")
                    nc.vector.reciprocal(rec[:], rs[:])
                    # normalize + bf16 cast in one ACT pass (per-partition
                    # scale AP); gpsimd.normalize_recip is ~2x slower and
                    # serializes with the gathers on GPSIMD
                    nc.scalar.mul(attn[:, n2, :], af[:], rec[:])
                attnT = atp.tile([128, 8, SEQ], dt.bfloat16, tag="attnT")
                attnTs[h] = attnT
                g = nc.gpsimd.dma_gather(
                    out_ap=attnT[:], in_ap=attn[:], idxs_ap=gidx[:],
                    num_idxs=SEQ, num_idxs_reg=SEQ, elem_size=TOT,
                    transpose=True, sbuf_tokens_per_rank=128,
                    sbuf_free_dim_per_rank=2 * TOT,
                    sbuf_free_dim_pad_per_rank=0, sbuf_byte_offset=0)
                add_dep_helper(g.ins, lib_inst.ins,
                               reason="dma_gather needs mlp gpsimd library")

            def emit_pv(h0):
                # PV for head pair (h0, h0+1), col-tiled into one PSUM tile
                hp = h0 // 2
                pvt = pvps.tile([128, SEQ], dt.float32, tag="pv")
                for par in range(2):
                    h = h0 + par
                    pb = 64 * par
                    for mc in range(8):
                        nc.tensor.matmul(
                            pvt[pb:pb + 64, :],
                            vv[:, mc, 64 * h:64 * (h + 1)],
                            attnTs[h][:, mc, :],
                            start=(mc == 0), stop=(mc == 7),
                            tile_position=(0, pb))
                    nc.vector.tensor_copy(avt[pb:pb + 64, hp, :],
                                          pvt[pb:pb + 64, :])

            def emit_out():
                for n2 in range(2):
                    psw = mps.tile([128, 1024], dt.float32, tag="m")
                    ps = psw[:, 0:512]
                    for c4 in range(4):
                        nc.tensor.matmul(ps,
                                         avt[:, c4, 128 * n2:128 * (n2 + 1)],
                                         wout[:, c4, :],
                                         start=(c4 == 0), stop=(c4 == 3))
                    osb = hsb.tile([128, 512], dt.float32, tag="osb")
                    nc.vector.tensor_add(osb[:], ps, bout[:])
                    nc.sync.dma_start(out_d.ap()[128 * n2:128 * (n2 + 1), :],
                                      osb[:])

            # ---------------- software-pipelined emission
            emit_qt()
            emit_bd(0); emit_bd(1)
            emit_kt((0, 1))          # covers h0/h1 round trip
            emit_bd(2); emit_bd(3)
            emit_ac(0); emit_ac(1)
            emit_kt((2, 3))
            emit_bd(4); emit_bd(5)
            emit_ac(2); emit_ac(3)
            emit_v((0, 2))
            emit_bd(6); emit_bd(7)
            emit_ac(4); emit_ac(5)
            emit_v((4, 6))
            emit_pv(0)
            emit_ac(6); emit_ac(7)
            emit_pv(2); emit_pv(4); emit_pv(6)
            emit_out()


# ---------------------------------------------------------------- host wrapper
_PROGRAM = None


def _get_program():
    global _PROGRAM
    if _PROGRAM is None:
        _PROGRAM = build_program()
    return _PROGRAM


def make_in_maps(x, memory, W_qkv, W_rel, W_out, b_out, u_emb, v_emb):
    x = np.asarray(x, dtype=F32)
    memory = np.asarray(memory, dtype=F32)
    W_qkv = np.asarray(W_qkv, dtype=F32)
    W_rel = np.asarray(W_rel, dtype=F32)
    W_out = np.asarray(W_out, dtype=F32)
    b_out = np.asarray(b_out, dtype=F32)
    u_emb = np.asarray(u_emb, dtype=F32)
    v_emb = np.asarray(v_emb, dtype=F32)

    R = _positional_encoding()                       # (1024, 22)
    rw = R @ W_rel                                   # (1024, 512)
    rwt = _chunked(np.ascontiguousarray(rw[::-1].T), 4).astype(BF16)

    wqkv = _chunked(W_qkv, 4).astype(BF16)           # (128, 4, 1536)
    wout = _chunked(W_out, 4).astype(BF16)           # (128, 4, 512)
    bout = np.tile(b_out[None, :], (128, 1)).astype(F32)
    u2 = np.tile(u_emb, 2)[:, None].astype(F32)
    v2 = np.tile(v_emb, 2)[:, None].astype(F32)
    p = np.arange(128)[:, None] % 16
    s = np.arange(16)[None, :]
    gidx = (s * 16 + p).astype(np.int16)             # (128, 16)
    ident = np.eye(128).astype(BF16)
    zpad = np.full((128, RSTRIDE - TOT), PAD, dtype=F32).astype(BF16)

    shared = dict(wqkv=wqkv, rwt=rwt, wout=wout, bout=bout,
                  u2=u2, v2=v2, gidx=gidx, ident=ident, zpad=zpad)
    in_maps = []
    for c in range(B):
        X = np.concatenate([memory[c], x[c]], axis=0)          # (1024, 512)
        xt = _chunked(np.ascontiguousarray(X.T), 4).astype(BF16)  # (128,4,1024)
        in_maps.append(dict(xt=xt, **shared))
    return in_maps


def run(in_maps, trace=False, **kw):
    nc = _get_program()
    res = run_bass_kernel_spmd(nc, in_maps, core_ids=list(range(B)),
                               trace=trace, **kw)
    out = np.stack([res.results[c]["out"] for c in range(B)]).astype(F32)
    return out, res


def kernel(x, memory, W_qkv, W_rel, W_out, b_out, u_emb, v_emb):
    in_maps = make_in_maps(x, memory, W_qkv, W_rel, W_out, b_out, u_emb, v_emb)
    out, _ = run(in_maps)
    return out.reshape(B, SEQ, DIM)
